# revision 54
# baseline (speedup 1.0000x reference)
"""nn_Attention: out[b,h] = strict_tril(rope(Q[b,h]) @ rope(Q[b,h])^T) @ V[b].

Sharding: one (b,h) pair per NeuronCore (B*H = 8 pairs on 8 cores, fully
data-parallel, no collectives).

Algorithm: blockwise *linear attention* with a running KV prefix state.
With QR = rope(Q) and S = strict_tril(QR QR^T):

    out[t] = sum_{s<t} <qr_t, qr_s> v_s = qr_t . KV_t,   KV_t = sum_{s<t} qr_s v_s^T

Processing t in waves of 256 rows (2 row blocks of 128):

    out_wave = QR_wave @ KV_state            (inter,  16 chunk matmuls x 256)
             + strict_tril(QR_wave QR_wave^T) @ V_wave   (intra, 3 blocks)
    KV_state += QR_wave^T @ V_wave           (update, 16 chunk matmuls)

This does ~2*T*N*D*2 + small-intra FLOPs instead of ~T*T*N (2x less PE
work than the masked-score formulation at T=2048, N=2048, D=256).

Layout trick: the host stages Q de-interleaved AND transposed
(qt = [Q[:,0::2]^T ; Q[:,1::2]^T], shape [n, t]) plus transposed cos/sin
tables [n/2, t].  RoPE then runs in the transposed domain where every
DVE op is a dense step-1 bf16 tensor_tensor (2x perf mode, no broadcast
APs), and its output IS QR^T in chunk layout - the lhsT for both the
score matmuls and the inter matmuls, with zero transposes.  Only the
KV-update needs row-major QR tiles (contraction over t), produced with
PE transpose-mode + one dense ACT copy per 8 chunks.

All device inputs are staged host-side in bf16 (the kernel computes in
bf16 regardless; the cast just moves to staging, halving HBM traffic)
and *wave-blocked*: hbm[p, wave, chunk, tau] so each per-partition DMA
source run is multi-KB -> line-rate descriptors.

Per-wave dataflow:
  DMA   : qt wave slice on the gpsimd ring; interleaved cos|sin wave
          slice on the sync ring (one DMA per wave); V / masks / output
          stores on the scalar ring (kept tiny so ACT never blocks).
  DVE   : 6 dense bf16 muls/adds (2x_1P mode; every SOURCE is a single
          dense run - multi-run sources drop the 2x mode) -> qrt cols.
  PE    : 2x16 transposes -> qr row tiles (ACT copies PSUM->SBUF),
          intra score strips (strict-diag masked on DVE), inter out,
          AV, KV update.  ~22 wide dummy matmuls up front hold the HAM
          clock gate at 8/8 before wave 0 lands.
  DVE   : KV flush adds (PSUM f32 + SBUF bf16 -> SBUF bf16), deferred
          past the next wave's RoPE in the DVE FIFO.
"""

import math
from functools import lru_cache

import numpy as np
import ml_dtypes

import concourse.bass as bass
import concourse.mybir as mybir
import concourse.tile as tile
from concourse import bacc
from concourse.bass_utils import run_bass_kernel_spmd

THETA = 2.0 ** 16
P = 128
TMODE = "pe"  # kept for test.py compat; unused

BF16 = mybir.dt.bfloat16
F32 = mybir.dt.float32


def _wave_block(a, t):
    """[rows, t] -> wave-blocked [128, nw * nchunks * 256] (pure relayout):
    out[p, w, c, tau] = a[c*128 + p, w*256 + tau]."""
    rows = a.shape[0]
    nch, nw = rows // 128, t // 256
    b = a.reshape(nch, 128, nw, 256).transpose(1, 2, 0, 3)
    return np.ascontiguousarray(b.reshape(128, nw * nch * 256))


@lru_cache(maxsize=None)
def _rope_tables(t, n):
    """Wave-blocked transposed cos/sin tables matching reference._rope, bf16."""
    idx = ((np.arange(n) // 2) * 2).astype(np.float32)
    freqs = (1.0 / (THETA ** (idx / np.float32(n))) / np.float32(2.0 * math.pi)).astype(
        np.float32
    )
    pos = np.arange(t, dtype=np.float32)[:, None]
    phases = ((pos * freqs) % np.float32(1.0)) * np.float32(2.0 * math.pi)
    cos_h = np.cos(phases)[:, 0::2]  # (t, n/2), one entry per pair
    sin_h = np.sin(phases)[:, 0::2]
    cwb = _wave_block(cos_h.T.astype(ml_dtypes.bfloat16), t)
    swb = _wave_block(sin_h.T.astype(ml_dtypes.bfloat16), t)
    nw, hpc = t // 256, n // 256
    cs = np.stack(
        [cwb.reshape(128, nw, hpc * 256), swb.reshape(128, nw, hpc * 256)], axis=2
    )
    return np.ascontiguousarray(cs.reshape(128, nw * 2 * hpc * 256))


@lru_cache(maxsize=None)
def _build(t, n, d, tmode="pe"):
    from contextlib import ExitStack

    nt = t // P          # row blocks
    nk = n // P          # contraction chunks
    hpc = n // 2 // P    # pair chunks per (even|odd) half
    tq = 2 * P           # rows per wave (W=2 row blocks)
    nw = t // tq         # number of waves
    assert n % (2 * P) == 0 and t % tq == 0 and d <= 512

    nw_, nk_, hpc_, tq_ = t // (2 * P), n // P, n // (2 * P), 2 * P
    nc = bacc.Bacc("TRN2", target_bir_lowering=False, debug=False, num_swdge_queues=4)
    # qt = [Q[:,0::2]^T ; Q[:,1::2]^T] staged bf16 (the kernel computes in
    # bf16 regardless - the cast just moves host-side, halving HBM traffic)
    # and wave-blocked [p, wave, chunk, tau] so every per-partition DMA
    # source run is (chunks*tq) contiguous -> multi-KB descriptors
    qt_d = nc.dram_tensor(
        "qt", [P, nw_ * nk_ * tq_], BF16, kind="ExternalInput"
    ).ap().rearrange("p (w c t) -> p w c t", w=nw_, c=nk_)
    v_d = nc.dram_tensor(
        "v", [P, (t // P) * d], BF16, kind="ExternalInput"
    ).ap().rearrange("p (j dd) -> p j dd", j=t // P)
    # cos/sin interleaved: one DMA per wave, 2*hpc*tq contiguous per prtn
    cs_d = nc.dram_tensor(
        "cs_t", [P, nw_ * 2 * hpc_ * tq_], BF16, kind="ExternalInput"
    ).ap().rearrange("p (w s c t) -> p w s c t", w=nw_, s=2, c=hpc_)
    um_d = nc.dram_tensor("umask_c", [P, P], BF16, kind="ExternalInput").ap()
    id_d = nc.dram_tensor("ident_c", [P, P], BF16, kind="ExternalInput").ap()
    out_d = nc.dram_tensor("out", [t, d], F32, kind="ExternalOutput").ap()

    with tile.TileContext(nc) as tc, ExitStack() as ctx:
        const = ctx.enter_context(tc.tile_pool(name="const", bufs=1))
        umask = const.tile([P, P], BF16, name="umask")
        ident = const.tile([P, P], BF16, name="ident")

        vpool = ctx.enter_context(tc.tile_pool(name="vpool", bufs=1))
        vb = vpool.tile([P, nt, d], BF16, name="vb")

        # QR^T chunk layout: chunk k ([n' in [kP,(k+1)P)] x [t]) at [:, k, :]
        qrt_pool = ctx.enter_context(tc.tile_pool(name="qrt_pool", bufs=1))
        qrt = qrt_pool.tile([P, nk, t], BF16, name="qrt")

        # KV state, bf16 (PSUM accumulates each wave's delta in f32; the
        # running cross-wave sum is kept in bf16 - ~8 rounded adds total)
        kv_pool = ctx.enter_context(tc.tile_pool(name="kv_pool", bufs=1))
        kvb = kv_pool.tile([P, nk, d], BF16, name="kvb")

        qtp = ctx.enter_context(tc.tile_pool(name="qtp", bufs=3))
        cpool = ctx.enter_context(tc.tile_pool(name="cpool", bufs=3))
        apool = ctx.enter_context(tc.tile_pool(name="apool", bufs=2))
        bpool = ctx.enter_context(tc.tile_pool(name="bpool", bufs=2))
        qrp = ctx.enter_context(tc.tile_pool(name="qrp", bufs=2))
        stp = ctx.enter_context(tc.tile_pool(name="stp", bufs=3))
        outp = ctx.enter_context(tc.tile_pool(name="outp", bufs=3))

        tpsum = ctx.enter_context(tc.tile_pool(name="tpsum", bufs=2, space="PSUM"))
        spsum = ctx.enter_context(tc.tile_pool(name="spsum", bufs=2, space="PSUM"))
        opsum = ctx.enter_context(tc.tile_pool(name="opsum", bufs=2, space="PSUM"))
        kpsum = ctx.enter_context(tc.tile_pool(name="kpsum", bufs=2, space="PSUM"))

        # ---- PE pre-warm: wide dummy matmuls lift the HAM clock gate
        # (4/8 -> 8/8) while wave-0 DMA + RoPE are in flight.  512-wide
        # keeps the array duty cycle high enough for the activity window.
        warm_src = const.tile([P, P], BF16, name="warm_src")
        warm_rhs = const.tile([P, 4 * P], BF16, name="warm_rhs")
        nc.vector.memset(warm_src, 0.0)
        nc.vector.memset(warm_rhs, 0.0)
        for ww in range(30):
            wtp = kpsum.tile([P, 4 * P], F32, tag="kvp", name=f"warm_{ww}")
            nc.tensor.matmul(wtp, lhsT=warm_src, rhs=warm_rhs, start=True, stop=True)

        pending_flush = []  # (chunk, kvp psum tile) awaiting add into kvb
        for w in range(nw):
            w0, w1 = w * tq, (w + 1) * tq
            i0, i1 = 2 * w, 2 * w + 1

            # ---- DMA: per-wave column slices -----------------------------
            # all q loads on the SWDGE ring (gpsimd has no other duties);
            # tables on sync; scalar/ACT only issues tiny V/mask/out DMAs
            qt_t = qtp.tile([P, nk, tq], BF16, tag="qt", name=f"qt_{w}")
            cst = cpool.tile([P, 2, hpc, tq], BF16, tag="ct", name=f"ct_{w}")
            ct_t = cst[:, 0]
            st_t = cst[:, 1]
            nc.gpsimd.dma_start(out=qt_t[:, 0:hpc, :], in_=qt_d[:, w, 0:hpc, :])
            nc.gpsimd.dma_start(
                out=qt_t[:, hpc:nk, :], in_=qt_d[:, w, hpc:nk, :]
            )
            nc.sync.dma_start(out=cst, in_=cs_d[:, w])
            if w == 0:
                # V + masks: wave-blocked bf16 on the scalar ring, first
                nc.scalar.dma_start(out=vb, in_=v_d)
                nc.scalar.dma_start(out=umask, in_=um_d)
                nc.scalar.dma_start(out=ident, in_=id_d)

            # ---- DVE RoPE -> qrt wave columns (all dense bf16, 2x) ------
            #   qrt[even chunk c] = qe_c*cos_c - qo_c*sin_c   (sub in place)
            #   qrt[odd  chunk c] = qo_c*cos_c + qe_c*sin_c   (add in place)
            rp, rcs = 1, hpc
            for g in range(rp):
                c0, c1 = g * rcs, (g + 1) * rcs
                qe = qt_t[:, c0:c1, :]
                qo = qt_t[:, hpc + c0 : hpc + c1, :]
                cc = ct_t[:, c0:c1, :]
                ss = st_t[:, c0:c1, :]
                # temps keep every DVE *source* AP a single dense run
                # (multi-run sources drop the 2x_1P perf mode)
                a1 = apool.tile([P, rcs, tq], BF16, tag="a1", name=f"a1_{w}_{g}")
                a2 = bpool.tile([P, rcs, tq], BF16, tag="a2", name=f"a2_{w}_{g}")
                nc.vector.tensor_mul(a1, qe, cc)
                nc.vector.tensor_mul(a2, qo, ss)
                nc.vector.tensor_sub(qrt[:, c0:c1, w0:w1], a1, a2)
                a3 = apool.tile([P, rcs, tq], BF16, tag="a1", name=f"a3_{w}_{g}")
                a4 = bpool.tile([P, rcs, tq], BF16, tag="a2", name=f"a4_{w}_{g}")
                nc.vector.tensor_mul(a3, qo, cc)
                nc.vector.tensor_mul(a4, qe, ss)
                nc.vector.tensor_add(qrt[:, hpc + c0 : hpc + c1, w0:w1], a3, a4)

            # ---- deferred KV flush from the previous wave ---------------
            # (emitted after this wave's RoPE so the DVE FIFO never blocks
            # next-wave RoPE behind PSUM waits on the KV-update matmuls)
            for kk, kvp in pending_flush:
                if w == 1:
                    nc.scalar.copy(kvb[:, kk : kk + 2, :], kvp)
                else:
                    nc.vector.tensor_add(
                        kvb[:, kk : kk + 2, :], kvb[:, kk : kk + 2, :], kvp
                    )
            pending_flush = []

            # ---- PE transposes: qrt chunks -> row-major QR tiles --------
            # (lhsT for the KV update; contraction there is over t)
            qr_ts = []
            for b, i in enumerate((i0, i1)):
                if w == nw - 1:
                    qr_ts.append(None)
                    break  # last wave never updates KV
                qr_t = qrp.tile([P, n], BF16, tag="qr", name=f"qr_{i}")
                for k0 in range(0, nk, 8):
                    nb = min(8, nk - k0)
                    tp = tpsum.tile([P, nb * P], BF16, tag="tp", name=f"tp_{i}_{k0}")
                    for j in range(nb):
                        nc.tensor.transpose(
                            tp[:, j * P : (j + 1) * P],
                            qrt[:, k0 + j, i * P : (i + 1) * P],
                            ident,
                        )
                    nc.scalar.copy(qr_t[:, k0 * P : (k0 + nb) * P], tp)
                qr_ts.append(qr_t)

            # ---- intra-wave score strips (upper-tri blocks, symmetric) --
            # strip0 = S(i0, cols[i0,i1]) (diag masked), strip1 = S(i1,i1)
            ps0 = spsum.tile([P, tq], F32, tag="ps", name=f"ps0_{w}")
            for k in range(nk):
                nc.tensor.matmul(
                    ps0,
                    lhsT=qrt[:, k, i0 * P : (i0 + 1) * P],
                    rhs=qrt[:, k, w0:w1],
                    start=(k == 0),
                    stop=(k == nk - 1),
                )
            ps1 = spsum.tile([P, P], F32, tag="ps", name=f"ps1_{w}")
            for k in range(nk):
                nc.tensor.matmul(
                    ps1,
                    lhsT=qrt[:, k, i1 * P : (i1 + 1) * P],
                    rhs=qrt[:, k, i1 * P : (i1 + 1) * P],
                    start=(k == 0),
                    stop=(k == nk - 1),
                )
            strip0 = stp.tile([P, tq], BF16, tag="s0", name=f"s0_{w}")
            strip1 = stp.tile([P, P], BF16, tag="s1", name=f"s1_{w}")
            nc.vector.tensor_mul(strip0[:, 0:P], ps0[:, 0:P], umask)
            nc.scalar.copy(strip0[:, P:tq], ps0[:, P:tq])
            nc.vector.tensor_mul(strip1, ps1, umask)

            # ---- out blocks: inter (QR_i @ KV) + intra (strips^T @ V) ---
            for b, i in enumerate((i0, i1)):
                po = opsum.tile([P, d], F32, tag="po", name=f"po_{i}")
                first = True
                if w > 0:
                    for k in range(nk):
                        nc.tensor.matmul(
                            po,
                            lhsT=qrt[:, k, i * P : (i + 1) * P],
                            rhs=kvb[:, k, :],
                            start=(k == 0),
                            stop=False,
                        )
                    first = False
                if b == 0:
                    nc.tensor.matmul(
                        po, lhsT=strip0[:, 0:P], rhs=vb[:, i0, :],
                        start=first, stop=True,
                    )
                else:
                    nc.tensor.matmul(
                        po, lhsT=strip0[:, P:tq], rhs=vb[:, i0, :],
                        start=first, stop=False,
                    )
                    nc.tensor.matmul(
                        po, lhsT=strip1, rhs=vb[:, i1, :],
                        start=False, stop=True,
                    )
                ot = outp.tile([P, d], F32, tag="ot", name=f"ot_{i}")
                nc.scalar.copy(ot, po)
                nc.scalar.dma_start(out=out_d[i * P : (i + 1) * P, :], in_=ot)

            # ---- KV update: KV += QR_wave^T @ V_wave (skip last wave) ---
            # flush of the PSUM deltas is deferred to after next wave's RoPE
            if w < nw - 1:
                for kk in range(0, nk, 2):
                    kvp = kpsum.tile([P, 2, d], F32, tag="kvp", name=f"kvp_{w}_{kk}")
                    for j in range(2):
                        k = kk + j
                        nc.tensor.matmul(
                            kvp[:, j, :],
                            lhsT=qr_ts[0][:, k * P : (k + 1) * P],
                            rhs=vb[:, i0, :],
                            start=True,
                            stop=False,
                        )
                        nc.tensor.matmul(
                            kvp[:, j, :],
                            lhsT=qr_ts[1][:, k * P : (k + 1) * P],
                            rhs=vb[:, i1, :],
                            start=False,
                            stop=True,
                        )
                    pending_flush.append((kk, kvp))

    nc.compile()
    return nc


def _run(Q, V, trace=False, **trace_kwargs):
    Q = np.asarray(Q, dtype=np.float32)
    V = np.asarray(V, dtype=np.float32)
    b, h, t, n = Q.shape
    d = V.shape[-1]
    ncores = b * h
    nc = _build(t, n, d)
    cs_t = _rope_tables(t, n)
    in_maps = []
    for core in range(ncores):
        bi, hi = divmod(core, h)
        qt = np.empty((n, t), dtype=ml_dtypes.bfloat16)
        qt[: n // 2] = Q[bi, hi][:, 0::2].T
        qt[n // 2 :] = Q[bi, hi][:, 1::2].T
        qt = _wave_block(qt, t)
        vwb = (
            V[bi, 0]
            .astype(ml_dtypes.bfloat16)
            .reshape(t // P, P, d)
            .transpose(1, 0, 2)
            .reshape(P, (t // P) * d)
        )
        in_maps.append(
            {
                "qt": qt,
                "v": np.ascontiguousarray(vwb),
                "cs_t": cs_t,
            }
        )
    um = np.triu(np.ones((128, 128), np.float32), 1).astype(ml_dtypes.bfloat16)
    ident = np.eye(128, dtype=ml_dtypes.bfloat16)
    for m in in_maps:
        m["umask_c"] = um
        m["ident_c"] = ident
    res = run_bass_kernel_spmd(
        nc, in_maps, core_ids=list(range(ncores)), trace=trace, **trace_kwargs
    )
    out = np.empty((b, h, t, d), dtype=np.float32)
    for core in range(ncores):
        bi, hi = divmod(core, h)
        out[bi, hi] = res.results[core]["out"]
    return out, res


def kernel(**inputs):
    out, _ = _run(inputs["Q"], inputs["V"], trace=False)
    return out
